# revision 7
# baseline (speedup 1.0000x reference)
"""Trainium2 Bass kernel for nn_AMPGCN (embedding_lookup + 3x BatchNorm/ReLU +
mean-pool + linear + log_softmax), distributed over 8 NeuronCores.

Algorithm
---------
Every activation x[n, (s,d)] through the BN/ReLU chain depends only on
(v=idx[n,s], s, d), because BatchNorm is a per-channel affine map. So instead
of materializing the [20000, 15360] activation tensor (1.2 GB), each core:

  1. builds per-position histograms CNT[s, v] of the sampled indices
     (one-hot outer-product matmuls on the TensorEngine),
  2. runs the BN chain on tiny per-position tables T_s [1433, 768]
     (fused tensor_tensor_reduce ops give the exact cnt-weighted batch
     statistics in one DVE pass per moment),
  3. collapses the linear head into per-position tables L_s = T_s @ W.T / S,
  4. all-gathers the L tables (3 waves of 8 positions) and recovers the
     7-float logit row per (node, position) with one-hot matmuls on the
     TensorEngine (select path), sums over positions, and applies
     log_softmax on the node shard.

Work is sharded over the position axis s (20 positions over 8 cores, 3 waves),
the final select + softmax over the node axis (2500 nodes/core).
"""
import sys

if "/opt/trn_rl_repo" not in sys.path:
    sys.path.insert(0, "/opt/trn_rl_repo")

import numpy as np
import ml_dtypes

import concourse.bass as bass
import concourse.tile as tile
from concourse import bacc, mybir
from concourse.bass import AP

# ---------------------------------------------------------------- constants
N, S, D, V, K = 20000, 20, 768, 1433, 7
EPS = 1e-5
NCORES = 8
P = 128
NLOC = N // NCORES            # 2500 nodes per core
NB = 20                       # node tiles per core (2560 padded)
NPAD = NB * P                 # 2560
TD = D // P                   # 6 d-tiles
SH = 3                        # s-slots per core (cores 4-7: slot 2 is dummy)
TN = 157                      # hist row-tiles per slot (ceil(20000/128))
NH = TN * P                   # 20096 padded hist rows
VA = 12                       # a = idx // 128 in [0, 12)
VPADD = 1536                  # VA * 128, padded vocab per slot in cnt scratch
VP = 1440                     # table vocab pitch (even -> DVE 2x modes)
GB = 16                       # hist batch size (tiles per vector instr)
ROWS_L = NCORES * V           # 11464 rows per gathered-wave table
F32 = mybir.dt.float32
BF16 = mybir.dt.bfloat16

# wave t gathers positions s = 8t + sw (sw = contributing core); widths:
WAVE_W = [8, 8, 4]
NSEL = 20                     # all positions go through the PE select path
GSEL = 5                      # node-tiles per select psum bank


def _build(level=5, reps=1):
    """Build + compile the SPMD Bass graph (identical on all 8 cores).

    level: pipeline prefix for hardware bisection
      1 = loads + histogram, 2 = +BN chain +L, 3 = +publish +collective,
      5 = full (select + softmax).
    """
    nc = bacc.Bacc("TRN2", target_bir_lowering=False, debug=False,
                   num_devices=NCORES)

    # ------------------------------------------------------------- params
    emb_t = nc.dram_tensor("emb_t", [D, V], BF16, kind="ExternalInput")
    # BN affine params, one [P, TD] block per (stage, slot)
    gvec = nc.dram_tensor("gvec", [3 * SH, P, TD], F32, kind="ExternalInput")
    bvec = nc.dram_tensor("bvec", [3 * SH, P, TD], F32, kind="ExternalInput")
    linw_t = nc.dram_tensor("linw_t", [P, TD, K], F32, kind="ExternalInput")
    linb_r = nc.dram_tensor("linb_r", [P, K], F32, kind="ExternalInput")
    hist_b = nc.dram_tensor("hist_b", [P, SH * TN], BF16, kind="ExternalInput")
    hist_a = nc.dram_tensor("hist_a", [P, SH * TN], BF16, kind="ExternalInput")
    iota128 = nc.dram_tensor("iota128", [P, P], BF16, kind="ExternalInput")
    arow = nc.dram_tensor("arow", [P, VA], BF16, kind="ExternalInput")
    # select-path inputs: one-hot of idx%128 per local node (b on partitions)
    # and idx//128 per (partition, node-tile), for every position s
    lohb = nc.dram_tensor("lohb", [NSEL, P, NPAD], BF16, kind="ExternalInput")
    lha = nc.dram_tensor("lha", [P, NSEL, NB], BF16, kind="ExternalInput")
    out_ext = nc.dram_tensor("out", [NLOC, K], F32, kind="ExternalOutput")

    # ------------------------------------------------------- DRAM internals
    cnt_dram = nc.dram_tensor("cnt_dram", [SH, VPADD], F32)
    ag_in = [nc.dram_tensor(f"ag_in{t}", [V * K], F32) for t in range(SH)]
    ag_out = [nc.dram_tensor(f"ag_out{t}", [ROWS_L * K], F32,
                             addr_space="Shared") for t in range(SH)]

    from contextlib import ExitStack
    with tile.TileContext(nc) as tc, ExitStack() as ctx:
      singles = ctx.enter_context(tc.tile_pool(name="singles", bufs=1))
      xpool = ctx.enter_context(tc.tile_pool(name="xpool", bufs=2))
      cntp = ctx.enter_context(tc.tile_pool(name="cntp", bufs=2))
      prodp = ctx.enter_context(tc.tile_pool(name="prodp", bufs=2))
      hb = ctx.enter_context(tc.tile_pool(name="hb", bufs=2))
      statp = ctx.enter_context(tc.tile_pool(name="statp", bufs=3))
      lp = ctx.enter_context(tc.tile_pool(name="lp", bufs=2))
      gp = ctx.enter_context(tc.tile_pool(name="gp", bufs=2))
      ph = ctx.enter_context(tc.tile_pool(name="ph", bufs=2, space="PSUM"))
      pl = ctx.enter_context(tc.tile_pool(name="pl", bufs=2, space="PSUM"))
      pq = ctx.enter_context(tc.tile_pool(name="pq", bufs=2, space="PSUM"))
      for _rep in range(reps):

        # ------------------------------------------------------ load inputs
        et_sb = singles.tile([P, TD, VP], BF16)       # E^T  [p, td, v]
        nc.vector.memset(et_sb[:, :, V:VP], 0.0)
        nc.sync.dma_start(
            out=et_sb[:, :, 0:V],
            in_=AP(tensor=emb_t, offset=0, ap=[[V, P], [P * V, TD], [1, V]]))

        g_sb = singles.tile([P, 3 * SH, TD], F32)
        nc.sync.dma_start(
            out=g_sb[:],
            in_=AP(tensor=gvec, offset=0,
                   ap=[[TD, P], [P * TD, 3 * SH], [1, TD]]))
        b_sb = singles.tile([P, 3 * SH, TD], F32)
        nc.sync.dma_start(
            out=b_sb[:],
            in_=AP(tensor=bvec, offset=0,
                   ap=[[TD, P], [P * TD, 3 * SH], [1, TD]]))

        lw_sb = singles.tile([P, TD, K], F32)
        nc.sync.dma_start(out=lw_sb[:], in_=linw_t.ap())
        lws_sb = singles.tile([P, TD, K], BF16)
        nc.scalar.mul(lws_sb[:], lw_sb[:], 1.0 / S)   # fold mean-pool 1/S

        lb_sb = singles.tile([P, K], F32)
        nc.sync.dma_start(out=lb_sb[:], in_=linb_r.ap())

        hb_sb = singles.tile([P, SH * TN], BF16)
        nc.sync.dma_start(out=hb_sb[:], in_=hist_b.ap())
        ha_sb = singles.tile([P, SH * TN], BF16)
        nc.sync.dma_start(out=ha_sb[:], in_=hist_a.ap())
        io_sb = singles.tile([P, P], BF16)
        nc.sync.dma_start(out=io_sb[:], in_=iota128.ap())
        ar_sb = singles.tile([P, VA], BF16)
        nc.sync.dma_start(out=ar_sb[:], in_=arow.ap())
        lha_sb = singles.tile([P, NSEL, NB], BF16)
        nc.sync.dma_start(out=lha_sb[:], in_=lha.ap())

        # per-(position, node-tile) select results, reduced over s at the end
        wsum = singles.tile([P, NSEL, NB, K], F32)
        if level < 5:
            nc.vector.memset(wsum[:], 0.0)
        eps_sb = singles.tile([P, 1], F32)
        nc.vector.memset(eps_sb[:], EPS)

        # ---------------------------------------------------- histogram (PE)
        cnt_rep = []   # per-slot CNT replicated across partitions [P, VP]
        for sl in range(SH):
            psum_h = ph.tile([P, VA], F32, space="PSUM")
            base = sl * TN
            done = 0
            while done < TN:
                gsz = min(GB, TN - done)
                oh = hb.tile([P, GB, P], BF16, tag="oh")
                nc.vector.tensor_tensor(
                    out=oh[:, :gsz, :],
                    in0=hb_sb[:, base + done:base + done + gsz]
                        .unsqueeze(2).to_broadcast([P, gsz, P]),
                    in1=io_sb[:].unsqueeze(1).to_broadcast([P, gsz, P]),
                    op=mybir.AluOpType.is_equal)
                mk = hb.tile([P, GB, VA], BF16, tag="mk")
                nc.vector.tensor_tensor(
                    out=mk[:, :gsz, :],
                    in0=ha_sb[:, base + done:base + done + gsz]
                        .unsqueeze(2).to_broadcast([P, gsz, VA]),
                    in1=ar_sb[:].unsqueeze(1).to_broadcast([P, gsz, VA]),
                    op=mybir.AluOpType.is_equal)
                for g in range(gsz):
                    nc.tensor.matmul(
                        out=psum_h[:],
                        lhsT=oh[:, g, :],
                        rhs=mk[:, g, :],
                        start=(done + g == 0),
                        stop=(done + g == TN - 1))
                done += gsz
            cnt_sb = statp.tile([P, VA], F32, tag="cnt_sb")
            nc.scalar.copy(cnt_sb[:], psum_h[:])
            # scatter to flat per-slot histogram: cnt_dram[sl, a*128 + b]
            nc.sync.dma_start(
                out=AP(tensor=cnt_dram, offset=sl * VPADD,
                       ap=[[1, P], [P, VA]]),
                in_=cnt_sb[:])
            # all-partition replica; [V:VP] is naturally zero (no index maps
            # there), which zero-pads every downstream product
            rep = cntp.tile([P, VP], F32, tag="cntrep")
            nc.sync.dma_start(
                out=rep[:],
                in_=AP(tensor=cnt_dram, offset=sl * VPADD,
                       ap=[[0, P], [1, VP]]))
            repb = cntp.tile([P, VP], BF16, tag="cntrepb")
            nc.vector.tensor_copy(repb[:], rep[:])
            cnt_rep.append(repb)

        # ------------------------------------------- per-slot BN-table chain
        for sl in range(SH if level >= 2 else 0):
            repb = cnt_rep[sl]
            x_cur = et_sb
            for stage in range(3):
                # cnt-weighted sums over v: ssum = sum cnt*x, ssq = sum cnt*x^2
                # Two-level reduction: 32-element chunks to bf16 partials in
                # the 2x DVE mode, then a short fp32 reduce (each partial
                # rounds once, so the bf16 error averages out over 45 chunks).
                ssum = statp.tile([P, TD], F32, tag="ssum")
                ssq = statp.tile([P, TD], F32, tag="ssq")
                prod = prodp.tile([P, TD, VP], BF16, tag="prod")
                nc.vector.tensor_mul(
                    prod[:], x_cur[:],
                    repb[:].unsqueeze(1).to_broadcast([P, TD, VP]))
                ps1 = statp.tile([P, TD, VP // 32], BF16, tag="ps1")
                with nc.allow_low_precision("bn stats: bf16 chunk partials"):
                    nc.vector.tensor_reduce(
                        out=ps1[:],
                        in_=prod[:].rearrange("p td (c w) -> p td c w", w=32),
                        axis=mybir.AxisListType.X, op=mybir.AluOpType.add)
                nc.vector.tensor_reduce(
                    out=ssum[:], in_=ps1[:],
                    axis=mybir.AxisListType.X, op=mybir.AluOpType.add)
                prod2 = prodp.tile([P, TD, VP], BF16, tag="prod2")
                nc.vector.tensor_mul(prod2[:], prod[:], x_cur[:])
                ps2 = statp.tile([P, TD, VP // 32], BF16, tag="ps2")
                with nc.allow_low_precision("bn stats: bf16 chunk partials"):
                    nc.vector.tensor_reduce(
                        out=ps2[:],
                        in_=prod2[:].rearrange("p td (c w) -> p td c w", w=32),
                        axis=mybir.AxisListType.X, op=mybir.AluOpType.add)
                nc.vector.tensor_reduce(
                    out=ssq[:], in_=ps2[:],
                    axis=mybir.AxisListType.X, op=mybir.AluOpType.add)
                if level < 2.2:
                    continue
                # stats -> affine params A, B  (all [P, TD])
                mu = statp.tile([P, TD], F32, tag="mu")
                nc.vector.tensor_scalar_mul(mu[:], ssum[:], 1.0 / N)
                e2 = statp.tile([P, TD], F32, tag="e2")
                nc.vector.tensor_scalar_mul(e2[:], ssq[:], 1.0 / N)
                mu2 = statp.tile([P, TD], F32, tag="mu2")
                nc.vector.tensor_mul(mu2[:], mu[:], mu[:])
                var = statp.tile([P, TD], F32, tag="var")
                nc.vector.tensor_sub(var[:], e2[:], mu2[:])
                sd = statp.tile([P, TD], F32, tag="sd")
                nc.scalar.activation(sd[:], var[:],
                                     mybir.ActivationFunctionType.Sqrt,
                                     bias=eps_sb[:])
                rinv = statp.tile([P, TD], F32, tag="rinv")
                nc.vector.reciprocal(rinv[:], sd[:])
                # ACT scale/bias operands need 64B-aligned offsets -> pad to
                # 16-f32 slots per td.
                aff_a = statp.tile([P, TD, 16], F32, tag="aff_a")
                nc.vector.tensor_mul(aff_a[:, :, 0:1], rinv[:].unsqueeze(2),
                                     g_sb[:, stage * SH + sl, :].unsqueeze(2))
                mua = statp.tile([P, TD], F32, tag="mua")
                nc.vector.tensor_mul(mua[:], mu[:], aff_a[:, :, 0])
                aff_b = statp.tile([P, TD, 16], F32, tag="aff_b")
                nc.vector.tensor_sub(
                    aff_b[:, :, 0:1],
                    b_sb[:, stage * SH + sl, :].unsqueeze(2),
                    mua[:].unsqueeze(2))
                if level < 2.4:
                    continue
                # x_next = relu(A * x + B)
                x_next = xpool.tile([P, TD, VP], BF16, tag="x")
                nc.vector.memset(x_next[:, :, V:VP], 0.0)
                for td in range(TD):
                    nc.scalar.activation(
                        x_next[:, td, 0:V], x_cur[:, td, 0:V],
                        mybir.ActivationFunctionType.Relu,
                        bias=aff_b[:, td, 0:1], scale=aff_a[:, td, 0:1])
                x_cur = x_next

            if level < 2.5:
                continue
            # ------------------------------- L_s = x4 @ (W.T / S)  [V, K]
            l_sb = lp.tile([P, VA, K], F32, tag="lsb")
            for vt in range(VA):
                vp = min(P, V - vt * P)
                psum_l = pl.tile([P, K], F32, space="PSUM", tag="psl")
                for td in range(TD):
                    nc.tensor.matmul(
                        out=psum_l[:vp, :],
                        lhsT=x_cur[:, td, vt * P:vt * P + vp],
                        rhs=lws_sb[:, td, :],
                        start=(td == 0), stop=(td == TD - 1))
                nc.scalar.copy(l_sb[:vp, vt, :], psum_l[:vp, :])
            if level < 3:
                continue
            # publish: ag_in[sl][(vt*128+p)*K + k] = l_sb[p, vt, k]
            nc.sync.dma_start(
                out=AP(tensor=ag_in[sl], offset=0,
                       ap=[[K, P], [P * K, VA - 1], [1, K]]),
                in_=l_sb[:, 0:VA - 1, :])
            vtail = V - (VA - 1) * P      # 25
            nc.sync.dma_start(
                out=AP(tensor=ag_in[sl], offset=(VA - 1) * P * K,
                       ap=[[K, vtail], [1, K]]),
                in_=l_sb[:vtail, VA - 1, :])

            # --------------------------- wave collective + select + reduce
            nc.gpsimd.collective_compute(
                "AllGather",
                mybir.AluOpType.bypass,
                replica_groups=[list(range(NCORES))],
                ins=[ag_in[sl].ap()],
                outs=[ag_out[sl].ap()],
            )
            if level < 5:
                continue
            # PE/DVE select path for this wave's positions:
            # Q[n,(a,k)] = sum_b onehot_b[b,n] * L[a*128+b, k], then
            # mask a = idx//128, reduce over a -> wsum[:, s, :, :]
            for sw in range(WAVE_W[sl]):
                s_glob = 8 * sl + sw
                lsf = lp.tile([P, VA, K], F32, tag="lsf")
                nc.vector.memset(lsf[:, VA - 1, :], 0.0)
                nc.sync.dma_start(
                    out=lsf[:, 0:VA - 1, :],
                    in_=AP(tensor=ag_out[sl], offset=sw * V * K,
                           ap=[[K, P], [P * K, VA - 1], [1, K]]))
                vt25 = V - (VA - 1) * P       # 25 rows in the last a-tile
                nc.sync.dma_start(
                    out=lsf[0:vt25, VA - 1, :],
                    in_=AP(tensor=ag_out[sl],
                           offset=(sw * V + (VA - 1) * P) * K,
                           ap=[[K, vt25], [1, K]]))
                lbs = lp.tile([P, VA * K], BF16, tag="lbs")
                nc.scalar.copy(lbs[:], lsf[:])
                ob = gp.tile([P, NPAD], BF16, tag="ob")
                nc.sync.dma_start(
                    out=ob[:],
                    in_=AP(tensor=lohb, offset=s_glob * P * NPAD,
                           ap=[[NPAD, P], [1, NPAD]]))
                am = gp.tile([P, NB, VA], BF16, tag="am")
                nc.vector.tensor_tensor(
                    out=am[:],
                    in0=lha_sb[:, s_glob, :].unsqueeze(2)
                        .to_broadcast([P, NB, VA]),
                    in1=ar_sb[:].unsqueeze(1).to_broadcast([P, NB, VA]),
                    op=mybir.AluOpType.is_equal)
                for g in range(NB // GSEL):
                    psq = pq.tile([P, GSEL, VA * K], F32, space="PSUM",
                                  tag="psq")
                    for t in range(GSEL):
                        tt = g * GSEL + t
                        nc.tensor.matmul(
                            out=psq[:, t, :],
                            lhsT=ob[:, tt * P:(tt + 1) * P],
                            rhs=lbs[:],
                            start=True, stop=True)
                    # sel[p, t, k, a] = psq[p, t, (a,k)] * am[p, t', a]
                    sel = gp.tile([P, GSEL, K, VA], BF16, tag="sel")
                    nc.vector.tensor_mul(
                        sel[:],
                        psq[:].rearrange("p t (a k) -> p t k a", k=K),
                        am[:, g * GSEL:(g + 1) * GSEL, :].unsqueeze(2)
                        .to_broadcast([P, GSEL, K, VA]))
                    nc.vector.tensor_reduce(
                        out=wsum[:, s_glob, g * GSEL:(g + 1) * GSEL, :],
                        in_=sel[:],
                        axis=mybir.AxisListType.X,
                        op=mybir.AluOpType.add)

        # ------------------------------------------- logits + log_softmax
        acc = singles.tile([P, NB, K], F32)
        nc.vector.tensor_reduce(
            out=acc[:],
            in_=wsum[:].rearrange("p s nb k -> p nb k s"),
            axis=mybir.AxisListType.X,
            op=mybir.AluOpType.add)
        nc.vector.tensor_add(acc[:], acc[:],
                             lb_sb[:].unsqueeze(1).to_broadcast([P, NB, K]))
        mx = singles.tile([P, NB], F32)
        nc.vector.tensor_reduce(out=mx[:], in_=acc[:],
                                axis=mybir.AxisListType.X,
                                op=mybir.AluOpType.max)
        xm = singles.tile([P, NB, K], F32)
        nc.vector.tensor_sub(xm[:], acc[:],
                             mx[:].unsqueeze(2).to_broadcast([P, NB, K]))
        ex = singles.tile([P, NB, K], F32)
        nc.scalar.activation(ex[:], xm[:], mybir.ActivationFunctionType.Exp)
        se = singles.tile([P, NB], F32)
        nc.vector.tensor_reduce(out=se[:], in_=ex[:],
                                axis=mybir.AxisListType.X,
                                op=mybir.AluOpType.add)
        ls = singles.tile([P, NB], F32)
        nc.scalar.activation(ls[:], se[:], mybir.ActivationFunctionType.Ln)
        res = singles.tile([P, NB, K], F32)
        nc.vector.tensor_sub(res[:], xm[:],
                             ls[:].unsqueeze(2).to_broadcast([P, NB, K]))

        # ------------------------------------------------------- output DMA
        # node n = nb*128 + p -> out row n (n < 2500)
        nc.sync.dma_start(
            out=AP(tensor=out_ext, offset=0,
                   ap=[[K, P], [P * K, NB - 1], [1, K]]),
            in_=res[:, 0:NB - 1, :])
        tail = NLOC - (NB - 1) * P    # 68
        nc.sync.dma_start(
            out=AP(tensor=out_ext, offset=(NB - 1) * P * K,
                   ap=[[K, tail], [1, K]]),
            in_=res[:tail, NB - 1, :])

    nc.compile()
    return nc


def _host_prep(inputs):
    """Pure layout marshalling of the (numpy) inputs into per-core maps."""
    idx = np.asarray(inputs["sampled_idx"], dtype=np.int32)
    E = np.asarray(inputs["emb_table"], dtype=np.float32)
    lin_w = np.asarray(inputs["lin_w"], dtype=np.float32)
    lin_b = np.asarray(inputs["lin_b"], dtype=np.float32)
    gs = [np.asarray(inputs[f"g{i}"], np.float32).reshape(S, D) for i in (1, 2, 3)]
    bs = [np.asarray(inputs[f"b{i}"], np.float32).reshape(S, D) for i in (1, 2, 3)]

    emb_t = np.ascontiguousarray(E.T).astype(ml_dtypes.bfloat16)  # [D, V]
    # lin_w.T arranged [p, td, k]
    lwt = lin_w.T.reshape(TD, P, K).transpose(1, 0, 2)      # [P, TD, K]
    linw_t = np.ascontiguousarray(lwt)
    linb_r = np.tile(lin_b[None, :], (P, 1))                # [P, K]
    iota128 = np.tile(np.arange(P, dtype=np.float32)[None, :], (P, 1)) \
        .astype(ml_dtypes.bfloat16)
    arow = np.tile(np.arange(VA, dtype=np.float32)[None, :], (P, 1)) \
        .astype(ml_dtypes.bfloat16)

    in_maps = []
    for c in range(NCORES):
        slots = [c, c + 8, c + 16 if c < 4 else -1]
        # g/b per (stage, slot): value g[s, td*128+p] laid out [P, TD]
        gv = np.zeros((3 * SH, P, TD), np.float32)
        bv = np.zeros((3 * SH, P, TD), np.float32)
        for st in range(3):
            for sl, s in enumerate(slots):
                if s < 0:
                    continue
                gv[st * SH + sl] = gs[st][s].reshape(TD, P).T
                bv[st * SH + sl] = bs[st][s].reshape(TD, P).T
        # hist columns
        hb_arr = np.full((P, SH * TN), -1.0, np.float32)
        ha_arr = np.full((P, SH * TN), -1.0, np.float32)
        for sl, s in enumerate(slots):
            if s < 0:
                continue
            col = np.full(NH, -1, np.int32)
            col[:N] = idx[:, s]
            cb = np.where(col >= 0, col % P, -1).astype(np.float32)
            ca = np.where(col >= 0, col // P, -1).astype(np.float32)
            hb_arr[:, sl * TN:(sl + 1) * TN] = cb.reshape(TN, P).T
            ha_arr[:, sl * TN:(sl + 1) * TN] = ca.reshape(TN, P).T

        # select path: one-hot of idx%128 per local node (b on partitions)
        # and idx//128 per (partition, node-tile), for all 20 positions
        idx_shard = idx[c * NLOC:(c + 1) * NLOC]            # [2500, S]
        lohb_np = np.zeros((NSEL, P, NPAD), np.float32)
        lha_np = np.full((P, NSEL, NB), -1.0, np.float32)
        ll = np.arange(NLOC)
        for s_glob in range(NSEL):
            vals = idx_shard[:, s_glob]                    # [2500]
            lohb_np[s_glob][vals % P, ll] = 1.0
            av = np.full(NPAD, -1.0, np.float32)
            av[:NLOC] = vals // P
            lha_np[:, s_glob, :] = av.reshape(NB, P).T

        in_map = {
            "emb_t": emb_t,
            "gvec": gv,
            "bvec": bv,
            "linw_t": linw_t,
            "linb_r": linb_r,
            "hist_b": hb_arr.astype(ml_dtypes.bfloat16),
            "hist_a": ha_arr.astype(ml_dtypes.bfloat16),
            "iota128": iota128,
            "arow": arow,
            "lohb": lohb_np.astype(ml_dtypes.bfloat16),
            "lha": lha_np.astype(ml_dtypes.bfloat16),
        }
        in_maps.append(in_map)
    return in_maps


_NC_CACHE = {}


def _get_nc(reps=1):
    key = ("nc", reps)
    if key not in _NC_CACHE:
        _NC_CACHE[key] = _build(reps=reps)
    return _NC_CACHE[key]


def _get_runner(reps=1):
    """Cached jitted SPMD executor (modeled on bass2jax.run_bass_via_pjrt,
    without buffer donation so the same device buffers can be re-executed
    for timing). Returns (fn, pack, unpack)."""
    rkey = ("runner", reps)
    if rkey in _NC_CACHE:
        return _NC_CACHE[rkey]
    import jax
    from jax.sharding import Mesh, PartitionSpec
    from jax.experimental.shard_map import shard_map
    from concourse import bass2jax

    nc = _get_nc(reps)
    bass2jax.install_neuronx_cc_hook()

    in_names, out_names, out_avals, zero_outs = [], [], [], []
    partition_name = (nc.partition_id_tensor.name
                      if nc.partition_id_tensor else None)
    for alloc in nc.m.functions[0].allocations:
        if not isinstance(alloc, mybir.MemoryLocationSet):
            continue
        name = alloc.memorylocations[0].name
        if alloc.kind == "ExternalInput":
            if name != partition_name:
                in_names.append(name)
        elif alloc.kind == "ExternalOutput":
            out_names.append(name)
            shape = tuple(alloc.tensor_shape)
            dtype = mybir.dt.np(alloc.dtype)
            out_avals.append(jax.core.ShapedArray(shape, dtype))
            zero_outs.append(np.zeros(shape, dtype))
    n_params = len(in_names)
    all_names = in_names + out_names
    if partition_name is not None:
        all_names.append(partition_name)

    def _body(*args):
        operands = list(args)
        if partition_name is not None:
            operands.append(bass2jax.partition_id_tensor())
        outs = bass2jax._bass_exec_p.bind(
            *operands,
            out_avals=tuple(out_avals),
            in_names=tuple(all_names),
            out_names=tuple(out_names),
            lowering_input_output_aliases=(),
            sim_require_finite=True,
            sim_require_nnan=True,
            nc=nc,
        )
        return tuple(outs)

    devices = jax.devices()[:NCORES]
    mesh = Mesh(np.asarray(devices), ("core",))
    n_outs = len(out_names)
    sharded = jax.jit(
        shard_map(_body, mesh=mesh,
                  in_specs=(PartitionSpec("core"),) * (n_params + n_outs),
                  out_specs=(PartitionSpec("core"),) * n_outs,
                  check_rep=False),
        keep_unused=True)

    def pack(in_maps):
        concat_in = [
            np.concatenate([np.asarray(in_maps[c][name])
                            for c in range(NCORES)], axis=0)
            for name in in_names
        ]
        concat_zeros = [
            np.zeros((NCORES * z.shape[0], *z.shape[1:]), z.dtype)
            for z in zero_outs
        ]
        return [jax.device_put(a) for a in concat_in + concat_zeros]

    def unpack(out_arrs):
        res = np.asarray(out_arrs[out_names.index("out")])
        return res.reshape(NCORES, NLOC, K)

    _NC_CACHE[rkey] = (sharded, pack, unpack)
    return _NC_CACHE[rkey]


def kernel(**inputs):
    fn, pack, unpack = _get_runner()
    args = pack(_host_prep(inputs))
    shards = unpack(fn(*args))
    return np.concatenate(list(shards), axis=0)


# revision 9
# speedup vs baseline: 29.5171x; 29.5171x over previous
"""Trainium2 Bass kernel for nn_AMPGCN (embedding_lookup + 3x BatchNorm/ReLU +
mean-pool + linear + log_softmax), distributed over 8 NeuronCores.

Algorithm
---------
Every activation x[n, (s,d)] through the BN/ReLU chain depends only on
(v=idx[n,s], s, d), because BatchNorm is a per-channel affine map. So instead
of materializing the [20000, 15360] activation tensor (1.2 GB), each core:

  1. builds per-position histograms CNT[s, v] of the sampled indices
     (one-hot outer-product matmuls on the TensorEngine),
  2. runs the BN chain on tiny per-position tables T_s [1433, 768];
     cnt-weighted batch stats come from z = sqrt(cnt)*x products (DVE)
     reduced by per-td DVE reduces (first moment) and ACT Square+accum
     (second moment),
  3. collapses the linear head into transposed per-position tables
     LT_s [7, 1433] (k-major, contiguous publish),
  4. all-gathers the LT tables (3 waves of 8 positions) and recovers the
     7 logits per (node, position) with one GpSimd ap_gather per position
     (table replicated across partition groups, k on partitions), sums
     positions, applies a partition-parallel log_softmax (PE selector
     matmuls for the k-sums), and writes the [128, 320] result which the
     host unpacks to [2500, 7] (pure layout).

Work is sharded over the position axis s (20 positions over 8 cores, 3
waves), the gather + softmax over the node axis (2500 nodes/core).
"""
import sys

if "/opt/trn_rl_repo" not in sys.path:
    sys.path.insert(0, "/opt/trn_rl_repo")

import numpy as np
import ml_dtypes

import concourse.bass as bass
import concourse.tile as tile
from concourse import bacc, mybir
from concourse.bass import AP

# ---------------------------------------------------------------- constants
N, S, D, V, K = 20000, 20, 768, 1433, 7
EPS = 1e-5
NCORES = 8
P = 128
NLOC = N // NCORES            # 2500 nodes per core
TD = D // P                   # 6 d-tiles
SH = 3                        # s-slots per core (cores 4-7: slot 2 is dummy)
TN = 157                      # hist row-tiles per slot (ceil(20000/128))
NH = TN * P                   # 20096 padded hist rows
VA = 12                       # a = idx // 128 in [0, 12)
VPADD = 1536                  # VA * 128, padded vocab per slot in cnt scratch
VP = 1440                     # table vocab pitch (even -> DVE 2x modes)
GB = 16                       # hist batch size (tiles per vector instr)
ROWS_L = NCORES * V           # 11464 rows per gathered-wave table
AGPAD = 16 * V                # replication-read overhang past the last block
F32 = mybir.dt.float32
BF16 = mybir.dt.bfloat16
I16 = mybir.dt.int16

WAVE_W = [8, 8, 4]            # wave t holds positions s = 8t + sw
NSEL = 20
GRP = 16                      # partitions per gpsimd core group
NG = 8                        # groups (Q7 cores)
NPG = 320                     # nodes per group (8*320 = 2560 padded nodes)
LCH = (512, 512, V - 1024)    # L-matmul moving chunks

# first-moment reduce placement knob: stages where ssum uses ACT Copy+accum
# instead of per-td DVE reduces (tune from profile)
SSUM_ON_ACT = ()


def _build(level=5, reps=1):
    """Build + compile the SPMD Bass graph (identical on all 8 cores).

    level: 1 = loads+hist, 2 = +BN chain, 3 = +L+publish+collective,
    5 = full (gather + softmax).
    """
    nc = bacc.Bacc("TRN2", target_bir_lowering=False, debug=False,
                   num_devices=NCORES)

    # ------------------------------------------------------------- params
    emb_t = nc.dram_tensor("emb_t", [D, V], BF16, kind="ExternalInput")
    gvec = nc.dram_tensor("gvec", [3 * SH, P, TD], F32, kind="ExternalInput")
    bvec = nc.dram_tensor("bvec", [3 * SH, P, TD], F32, kind="ExternalInput")
    linw_t = nc.dram_tensor("linw_t", [P, TD, K], F32, kind="ExternalInput")
    linb_c = nc.dram_tensor("linb_c", [P, 1], F32, kind="ExternalInput")
    hist_b = nc.dram_tensor("hist_b", [P, SH * TN], BF16, kind="ExternalInput")
    hist_a = nc.dram_tensor("hist_a", [P, SH * TN], BF16, kind="ExternalInput")
    iota128 = nc.dram_tensor("iota128", [P, P], BF16, kind="ExternalInput")
    arow = nc.dram_tensor("arow", [P, VA], BF16, kind="ExternalInput")
    ghx = nc.dram_tensor("ghx", [P, NSEL, NPG // GRP], I16,
                         kind="ExternalInput")
    sel7 = nc.dram_tensor("sel7", [P, NG], F32, kind="ExternalInput")
    sel7t = nc.dram_tensor("sel7t", [NG, P], F32, kind="ExternalInput")
    out_ext = nc.dram_tensor("out", [P, NPG], F32, kind="ExternalOutput")

    # ------------------------------------------------------- DRAM internals
    cnt_dram = nc.dram_tensor("cnt_dram", [SH, VPADD], F32)
    ag_in = [nc.dram_tensor(f"ag_in{t}", [K * V], F32) for t in range(SH)]
    ag_out = [nc.dram_tensor(f"ag_out{t}", [ROWS_L * K + AGPAD], F32,
                             addr_space="Shared") for t in range(SH)]

    from contextlib import ExitStack
    with tile.TileContext(nc) as tc, ExitStack() as ctx:
      singles = ctx.enter_context(tc.tile_pool(name="singles", bufs=1))
      xpool = ctx.enter_context(tc.tile_pool(name="xpool", bufs=4))
      cntp = ctx.enter_context(tc.tile_pool(name="cntp", bufs=2))
      sqp = ctx.enter_context(tc.tile_pool(name="sqp", bufs=3))
      zp = ctx.enter_context(tc.tile_pool(name="zp", bufs=4))
      prp = ctx.enter_context(tc.tile_pool(name="prp", bufs=4))
      jkp = ctx.enter_context(tc.tile_pool(name="jkp", bufs=2))
      hb = ctx.enter_context(tc.tile_pool(name="hb", bufs=2))
      statp = ctx.enter_context(tc.tile_pool(name="statp", bufs=3))
      ltp = ctx.enter_context(tc.tile_pool(name="ltp", bufs=2))
      gp = ctx.enter_context(tc.tile_pool(name="gp", bufs=3))
      ph = ctx.enter_context(tc.tile_pool(name="ph", bufs=2, space="PSUM"))
      plt = ctx.enter_context(tc.tile_pool(name="plt", bufs=2, space="PSUM"))
      psm = ctx.enter_context(tc.tile_pool(name="psm", bufs=2, space="PSUM"))
      for _rep in range(reps):

        # ------------------------------------------------------ load inputs
        et_sb = singles.tile([P, TD, VP], BF16)       # E^T  [p, td, v]
        nc.vector.memset(et_sb[:, :, V:VP], 0.0)
        nc.sync.dma_start(
            out=et_sb[:, :, 0:V],
            in_=AP(tensor=emb_t, offset=0, ap=[[V, P], [P * V, TD], [1, V]]))

        g_sb = singles.tile([P, 3 * SH, TD], F32)
        nc.sync.dma_start(
            out=g_sb[:],
            in_=AP(tensor=gvec, offset=0,
                   ap=[[TD, P], [P * TD, 3 * SH], [1, TD]]))
        b_sb = singles.tile([P, 3 * SH, TD], F32)
        nc.sync.dma_start(
            out=b_sb[:],
            in_=AP(tensor=bvec, offset=0,
                   ap=[[TD, P], [P * TD, 3 * SH], [1, TD]]))

        lw_sb = singles.tile([P, TD, K], F32)
        nc.sync.dma_start(out=lw_sb[:], in_=linw_t.ap())
        lws_sb = singles.tile([P, TD, K], BF16)
        nc.scalar.mul(lws_sb[:], lw_sb[:], 1.0 / S)   # fold mean-pool 1/S

        lb_sb = singles.tile([P, 1], F32)
        nc.sync.dma_start(out=lb_sb[:], in_=linb_c.ap())

        hb_sb = singles.tile([P, SH * TN], BF16)
        nc.sync.dma_start(out=hb_sb[:], in_=hist_b.ap())
        ha_sb = singles.tile([P, SH * TN], BF16)
        nc.sync.dma_start(out=ha_sb[:], in_=hist_a.ap())
        io_sb = singles.tile([P, P], BF16)
        nc.sync.dma_start(out=io_sb[:], in_=iota128.ap())
        ar_sb = singles.tile([P, VA], BF16)
        nc.sync.dma_start(out=ar_sb[:], in_=arow.ap())
        ghx_sb = singles.tile([P, NSEL, NPG // GRP], I16)
        nc.sync.dma_start(out=ghx_sb[:], in_=ghx.ap())
        s7_sb = singles.tile([P, NG], F32)
        nc.sync.dma_start(out=s7_sb[:], in_=sel7.ap())
        s7t_sb = singles.tile([NG, P], F32)
        nc.sync.dma_start(out=s7t_sb[:], in_=sel7t.ap())

        wsum2 = singles.tile([P, NPG], F32)
        nc.vector.memset(wsum2[:], 0.0)
        eps_sb = singles.tile([P, 1], F32)
        nc.vector.memset(eps_sb[:], EPS)

        # zero the replication-read overhang past each wave's last block
        zpad = singles.tile([P, AGPAD // P], F32)
        nc.vector.memset(zpad[:], 0.0)
        for t in range(SH):
            nc.sync.dma_start(
                out=AP(tensor=ag_out[t], offset=ROWS_L * K,
                       ap=[[AGPAD // P, P], [1, AGPAD // P]]),
                in_=zpad[:])

        # ---------------------------------------------------- histogram (PE)
        sqcs = [None] * SH

        def emit_hist(sl):
            psum_h = ph.tile([P, VA], F32, space="PSUM", tag="ph")
            base = sl * TN
            done = 0
            while done < TN:
                gsz = min(GB, TN - done)
                oh = hb.tile([P, GB, P], BF16, tag="oh")
                nc.vector.tensor_tensor(
                    out=oh[:, :gsz, :],
                    in0=hb_sb[:, base + done:base + done + gsz]
                        .unsqueeze(2).to_broadcast([P, gsz, P]),
                    in1=io_sb[:].unsqueeze(1).to_broadcast([P, gsz, P]),
                    op=mybir.AluOpType.is_equal)
                mk = hb.tile([P, GB, VA], BF16, tag="mk")
                nc.vector.tensor_tensor(
                    out=mk[:, :gsz, :],
                    in0=ha_sb[:, base + done:base + done + gsz]
                        .unsqueeze(2).to_broadcast([P, gsz, VA]),
                    in1=ar_sb[:].unsqueeze(1).to_broadcast([P, gsz, VA]),
                    op=mybir.AluOpType.is_equal)
                for g in range(gsz):
                    nc.tensor.matmul(
                        out=psum_h[:],
                        lhsT=oh[:, g, :],
                        rhs=mk[:, g, :],
                        start=(done + g == 0),
                        stop=(done + g == TN - 1))
                done += gsz
            cnt_sb = statp.tile([P, VA], F32, tag="cnt_sb")
            nc.scalar.copy(cnt_sb[:], psum_h[:])
            # scatter to flat per-slot histogram: cnt_dram[sl, a*128 + b];
            # [V:VP] stays zero (no index maps there) which zero-pads
            # every downstream product
            nc.sync.dma_start(
                out=AP(tensor=cnt_dram, offset=sl * VPADD,
                       ap=[[1, P], [P, VA]]),
                in_=cnt_sb[:])
            rep = cntp.tile([P, VP], F32, tag="cntrep")
            nc.sync.dma_start(
                out=rep[:],
                in_=AP(tensor=cnt_dram, offset=sl * VPADD,
                       ap=[[0, P], [1, VP]]))
            sqc = sqp.tile([P, VP], BF16, tag="sqc")
            nc.scalar.activation(sqc[:], rep[:],
                                 mybir.ActivationFunctionType.Sqrt)
            sqcs[sl] = sqc

        # ------------------------------------------- per-slot BN-table chain
        x_curs = [et_sb] * SH
        affs = [None] * SH   # (ssum, ssq) tiles in flight per slot

        def emit_moments(sl, stage):
            sqc = sqcs[sl]
            x = x_curs[sl]
            ssum = statp.tile([P, TD], F32, tag=f"ssum{sl}")
            ssq = statp.tile([P, TD], F32, tag=f"ssq{sl}")
            zs, prods = [], []
            for td in range(TD):
                z = zp.tile([P, VP], BF16, tag="z")
                nc.vector.tensor_mul(z[:], sqc[:], x[:, td, :])
                pr = prp.tile([P, VP], BF16, tag="prod")
                nc.vector.tensor_mul(pr[:], sqc[:], z[:])
                zs.append(z)
                prods.append(pr)
            for td in range(TD):
                if stage in SSUM_ON_ACT:
                    jk = jkp.tile([P, VP], BF16, tag="jka")
                    nc.scalar.activation(
                        jk[:], prods[td][:],
                        mybir.ActivationFunctionType.Copy,
                        accum_out=ssum[:, td:td + 1])
                else:
                    nc.vector.tensor_reduce(
                        out=ssum[:, td:td + 1], in_=prods[td][:],
                        axis=mybir.AxisListType.X, op=mybir.AluOpType.add)
            for td in range(TD):
                jk = jkp.tile([P, VP], BF16, tag="jkb")
                nc.scalar.activation(
                    jk[:], zs[td][:],
                    mybir.ActivationFunctionType.Square,
                    accum_out=ssq[:, td:td + 1])
            affs[sl] = (ssum, ssq)

        def emit_stats_affine(sl, stage):
            ssum, ssq = affs[sl]
            mu = statp.tile([P, TD], F32, tag="mu")
            nc.vector.tensor_scalar_mul(mu[:], ssum[:], 1.0 / N)
            e2 = statp.tile([P, TD], F32, tag="e2")
            nc.vector.tensor_scalar_mul(e2[:], ssq[:], 1.0 / N)
            mu2 = statp.tile([P, TD], F32, tag="mu2")
            nc.vector.tensor_mul(mu2[:], mu[:], mu[:])
            var = statp.tile([P, TD], F32, tag="var")
            nc.vector.tensor_sub(var[:], e2[:], mu2[:])
            sd = statp.tile([P, TD], F32, tag="sd")
            nc.scalar.activation(sd[:], var[:],
                                 mybir.ActivationFunctionType.Sqrt,
                                 bias=eps_sb[:])
            rinv = statp.tile([P, TD], F32, tag="rinv")
            nc.vector.reciprocal(rinv[:], sd[:])
            # ACT scale/bias operands need 64B-aligned offsets -> pad to
            # 16-f32 slots per td.
            aff_a = statp.tile([P, TD, 16], F32, tag="aff_a")
            nc.vector.tensor_mul(aff_a[:, :, 0:1], rinv[:].unsqueeze(2),
                                 g_sb[:, stage * SH + sl, :].unsqueeze(2))
            mua = statp.tile([P, TD], F32, tag="mua")
            nc.vector.tensor_mul(mua[:], mu[:], aff_a[:, :, 0])
            aff_b = statp.tile([P, TD, 16], F32, tag="aff_b")
            nc.vector.tensor_sub(
                aff_b[:, :, 0:1],
                b_sb[:, stage * SH + sl, :].unsqueeze(2),
                mua[:].unsqueeze(2))
            affs[sl] = (aff_a, aff_b)

        def emit_relu(sl, stage):
            aff_a, aff_b = affs[sl]
            x = x_curs[sl]
            x_next = xpool.tile([P, TD, VP], BF16, tag="x")
            nc.vector.memset(x_next[:, :, V:VP], 0.0)
            for td in range(TD):
                nc.scalar.activation(
                    x_next[:, td, 0:V], x[:, td, 0:V],
                    mybir.ActivationFunctionType.Relu,
                    bias=aff_b[:, td, 0:1], scale=aff_a[:, td, 0:1])
            x_curs[sl] = x_next

        def emit_l_publish(sl):
            # LT_s = (W/S).T-contracted tables, k-major [7, 1433]
            x4 = x_curs[sl]
            lt7 = ltp.tile([K, V], F32, tag="lt7")
            c0 = 0
            for cw in LCH:
                psl = plt.tile([K, 512], F32, space="PSUM", tag="psl")
                for td in range(TD):
                    nc.tensor.matmul(
                        out=psl[:, 0:cw],
                        lhsT=lws_sb[:, td, :],
                        rhs=x4[:, td, c0:c0 + cw],
                        start=(td == 0), stop=(td == TD - 1))
                nc.scalar.copy(lt7[:, c0:c0 + cw], psl[:, 0:cw])
                c0 += cw
            nc.sync.dma_start(
                out=AP(tensor=ag_in[sl], offset=0, ap=[[V, K], [1, V]]),
                in_=lt7[:])
            nc.gpsimd.collective_compute(
                "AllGather",
                mybir.AluOpType.bypass,
                replica_groups=[list(range(NCORES))],
                ins=[ag_in[sl].ap()],
                outs=[AP(tensor=ag_out[sl], offset=0,
                         ap=[[1, ROWS_L * K]])],
            )

        def emit_gather_wave(sl):
            for sw in range(WAVE_W[sl]):
                s_glob = 8 * sl + sw
                tblr = gp.tile([P, V], F32, tag="tblr")
                nc.sync.dma_start(
                    out=tblr[:],
                    in_=AP(tensor=ag_out[sl], offset=sw * K * V,
                           ap=[[0, NG], [V, GRP], [1, V]]))
                gout = gp.tile([P, NPG], F32, tag="gout")
                nc.gpsimd.ap_gather(
                    out_ap=gout[:].unsqueeze(2),
                    in_ap=tblr[:].unsqueeze(2),
                    idxs_ap=ghx_sb[:, s_glob, :],
                    channels=P, num_elems=V, d=1, num_idxs=NPG)
                nc.vector.tensor_add(wsum2[:], wsum2[:], gout[:])

        # ---------------------------------------------- pipelined emission
        # slots skewed by 2 stages: round r runs (sl, r - 2sl)
        emit_hist(0)
        pending = []
        for r in range(7):
            work = [(sl, r - 2 * sl) for sl in range(SH)
                    if 0 <= r - 2 * sl < 3]
            if level >= 2:
                for sl, st in work:
                    emit_moments(sl, st)
            if r == 0:
                emit_hist(1)
            if r == 2:
                emit_hist(2)
            if level >= 2:
                for sl, st in work:
                    emit_stats_affine(sl, st)
                for sl in pending:
                    if level >= 5 or (level >= 4 and sl == 0) or (
                            level >= 4.5 and sl <= 1):
                        emit_gather_wave(sl)
                pending = []
                for sl, st in work:
                    emit_relu(sl, st)
                    if st == 2 and level >= 3:
                        emit_l_publish(sl)
                        pending.append(sl)
        for sl in pending:
            if level >= 5 or (level >= 4 and sl == 0) or (
                    level >= 4.5 and sl <= 1):
                emit_gather_wave(sl)

        # --------------------------- log_softmax (k on partition groups)
        # logits = wsum2 + b_k; exp on ACT (|logit| is O(1): no max shift)
        ex = singles.tile([P, NPG], F32)
        nc.scalar.activation(ex[:], wsum2[:],
                             mybir.ActivationFunctionType.Exp,
                             bias=lb_sb[:, 0:1])
        se_ps = psm.tile([NG, NPG], F32, space="PSUM", tag="se")
        nc.tensor.matmul(out=se_ps[:], lhsT=s7_sb[:], rhs=ex[:],
                         start=True, stop=True)
        ls = singles.tile([NG, NPG], F32)
        nc.scalar.activation(ls[:], se_ps[:],
                             mybir.ActivationFunctionType.Ln)
        lsb_ps = psm.tile([P, NPG], F32, space="PSUM", tag="lsb")
        nc.tensor.matmul(out=lsb_ps[:], lhsT=s7t_sb[:], rhs=ls[:],
                         start=True, stop=True)
        res = singles.tile([P, NPG], F32)
        nc.vector.scalar_tensor_tensor(
            out=res[:], in0=wsum2[:], scalar=lb_sb[:, 0:1], in1=lsb_ps[:],
            op0=mybir.AluOpType.add, op1=mybir.AluOpType.subtract)

        # host unpacks [128, 320] -> [2500, 7] (pure layout)
        nc.sync.dma_start(out=out_ext.ap(), in_=res[:])

    nc.compile()
    return nc


def _host_prep(inputs):
    """Pure layout marshalling of the (numpy) inputs into per-core maps."""
    idx = np.asarray(inputs["sampled_idx"], dtype=np.int32)
    E = np.asarray(inputs["emb_table"], dtype=np.float32)
    lin_w = np.asarray(inputs["lin_w"], dtype=np.float32)
    lin_b = np.asarray(inputs["lin_b"], dtype=np.float32)
    gs = [np.asarray(inputs[f"g{i}"], np.float32).reshape(S, D) for i in (1, 2, 3)]
    bs = [np.asarray(inputs[f"b{i}"], np.float32).reshape(S, D) for i in (1, 2, 3)]

    emb_t = np.ascontiguousarray(E.T).astype(ml_dtypes.bfloat16)  # [D, V]
    lwt = lin_w.T.reshape(TD, P, K).transpose(1, 0, 2)      # [P, TD, K]
    linw_t = np.ascontiguousarray(lwt)
    linb_c = np.zeros((P, 1), np.float32)
    for p in range(P):
        if p % GRP < K:
            linb_c[p, 0] = lin_b[p % GRP]
    iota128 = np.tile(np.arange(P, dtype=np.float32)[None, :], (P, 1)) \
        .astype(ml_dtypes.bfloat16)
    arow = np.tile(np.arange(VA, dtype=np.float32)[None, :], (P, 1)) \
        .astype(ml_dtypes.bfloat16)
    sel7 = np.zeros((P, NG), np.float32)
    sel7t = np.zeros((NG, P), np.float32)
    for p in range(P):
        g = p // GRP
        sel7t[g, p] = 1.0
        if p % GRP < K:
            sel7[p, g] = 1.0

    in_maps = []
    for c in range(NCORES):
        slots = [c, c + 8, c + 16 if c < 4 else -1]
        gv = np.zeros((3 * SH, P, TD), np.float32)
        bv = np.zeros((3 * SH, P, TD), np.float32)
        for st in range(3):
            for sl, s in enumerate(slots):
                if s < 0:
                    continue
                gv[st * SH + sl] = gs[st][s].reshape(TD, P).T
                bv[st * SH + sl] = bs[st][s].reshape(TD, P).T
        hb_arr = np.full((P, SH * TN), -1.0, np.float32)
        ha_arr = np.full((P, SH * TN), -1.0, np.float32)
        for sl, s in enumerate(slots):
            if s < 0:
                continue
            col = np.full(NH, -1, np.int32)
            col[:N] = idx[:, s]
            cb = np.where(col >= 0, col % P, -1).astype(np.float32)
            ca = np.where(col >= 0, col // P, -1).astype(np.float32)
            hb_arr[:, sl * TN:(sl + 1) * TN] = cb.reshape(TN, P).T
            ha_arr[:, sl * TN:(sl + 1) * TN] = ca.reshape(TN, P).T

        # gather indices: group g handles nodes [g*320, (g+1)*320); flat
        # order inside a group is i = s16*16 + p16 (ap_gather unwrap order)
        idx_shard = idx[c * NLOC:(c + 1) * NLOC]            # [2500, S]
        ghx_np = np.zeros((P, NSEL, NPG // GRP), np.int16)
        for g in range(NG):
            for i in range(NPG):
                node = g * NPG + i
                if node >= NLOC:
                    continue
                p16, s16 = i % GRP, i // GRP
                ghx_np[g * GRP + p16, :, s16] = idx_shard[node, :]

        in_map = {
            "emb_t": emb_t,
            "gvec": gv,
            "bvec": bv,
            "linw_t": linw_t,
            "linb_c": linb_c,
            "hist_b": hb_arr.astype(ml_dtypes.bfloat16),
            "hist_a": ha_arr.astype(ml_dtypes.bfloat16),
            "iota128": iota128,
            "arow": arow,
            "ghx": ghx_np,
            "sel7": sel7,
            "sel7t": sel7t,
        }
        in_maps.append(in_map)
    return in_maps


def _unpack_core(arr):
    """[128, 320] device layout -> [2500, 7] (row 16g+j, col i = node
    g*320+i, class j)."""
    a = np.asarray(arr).reshape(NG, GRP, NPG)[:, :K, :]     # [8, 7, 320]
    return a.transpose(0, 2, 1).reshape(NG * NPG, K)[:NLOC]


_NC_CACHE = {}


def _get_nc(reps=1):
    key = ("nc", reps)
    if key not in _NC_CACHE:
        _NC_CACHE[key] = _build(reps=reps)
    return _NC_CACHE[key]


def _get_runner(reps=1):
    """Cached jitted SPMD executor (modeled on bass2jax.run_bass_via_pjrt,
    without buffer donation so the same device buffers can be re-executed
    for timing). Returns (fn, pack, unpack)."""
    rkey = ("runner", reps)
    if rkey in _NC_CACHE:
        return _NC_CACHE[rkey]
    import jax
    from jax.sharding import Mesh, PartitionSpec
    from jax.experimental.shard_map import shard_map
    from concourse import bass2jax

    nc = _get_nc(reps)
    bass2jax.install_neuronx_cc_hook()

    in_names, out_names, out_avals, zero_outs = [], [], [], []
    partition_name = (nc.partition_id_tensor.name
                      if nc.partition_id_tensor else None)
    for alloc in nc.m.functions[0].allocations:
        if not isinstance(alloc, mybir.MemoryLocationSet):
            continue
        name = alloc.memorylocations[0].name
        if alloc.kind == "ExternalInput":
            if name != partition_name:
                in_names.append(name)
        elif alloc.kind == "ExternalOutput":
            out_names.append(name)
            shape = tuple(alloc.tensor_shape)
            dtype = mybir.dt.np(alloc.dtype)
            out_avals.append(jax.core.ShapedArray(shape, dtype))
            zero_outs.append(np.zeros(shape, dtype))
    n_params = len(in_names)
    all_names = in_names + out_names
    if partition_name is not None:
        all_names.append(partition_name)

    def _body(*args):
        operands = list(args)
        if partition_name is not None:
            operands.append(bass2jax.partition_id_tensor())
        outs = bass2jax._bass_exec_p.bind(
            *operands,
            out_avals=tuple(out_avals),
            in_names=tuple(all_names),
            out_names=tuple(out_names),
            lowering_input_output_aliases=(),
            sim_require_finite=True,
            sim_require_nnan=True,
            nc=nc,
        )
        return tuple(outs)

    devices = jax.devices()[:NCORES]
    mesh = Mesh(np.asarray(devices), ("core",))
    n_outs = len(out_names)
    sharded = jax.jit(
        shard_map(_body, mesh=mesh,
                  in_specs=(PartitionSpec("core"),) * (n_params + n_outs),
                  out_specs=(PartitionSpec("core"),) * n_outs,
                  check_rep=False),
        keep_unused=True)

    def pack(in_maps):
        concat_in = [
            np.concatenate([np.asarray(in_maps[c][name])
                            for c in range(NCORES)], axis=0)
            for name in in_names
        ]
        concat_zeros = [
            np.zeros((NCORES * z.shape[0], *z.shape[1:]), z.dtype)
            for z in zero_outs
        ]
        return [jax.device_put(a) for a in concat_in + concat_zeros]

    def unpack(out_arrs):
        res = np.asarray(out_arrs[out_names.index("out")])
        return res.reshape(NCORES, P, NPG)

    _NC_CACHE[rkey] = (sharded, pack, unpack)
    return _NC_CACHE[rkey]


def kernel(**inputs):
    fn, pack, unpack = _get_runner()
    args = pack(_host_prep(inputs))
    shards = unpack(fn(*args))
    return np.concatenate([_unpack_core(shards[c]) for c in range(NCORES)],
                          axis=0)
